# revision 6
# baseline (speedup 1.0000x reference)
"""KNRM ranking kernel for 8 Trainium2 NeuronCores.

Data-parallel over batch (1024 -> 8 x 128). Per core:
  - gather pre-normalized embeddings for query/doc token ids (indirect DMA)
  - PE-transpose gathered tiles so the embed dim is on partitions
  - cosine sim = matmul of normalized embeddings (simT layout: [d, (b,q)])
  - soft histogram: exp(-(s-mu_k)^2/(2 sigma_k^2)) for 11 kernels, factorized
    as U(s)*V_k(s) with U = exp(-50 s^2), V_k = exp(100 mu_k s - 50 mu_k^2)
    for the sigma=0.1 bins; the exact bin (mu=1, sigma=0.001) done directly.
  - sum over doc dim via PE ones-selector matmuls into PSUM, log1p via ACT
    Log(bias=1), MLP dot via PE, query-sum via DVE reduce, sigmoid via exp+recip.
"""

import os
from contextlib import ExitStack

import numpy as np

LAST_RESULT = None

B, QLEN, DLEN, EMBED, VOCAB, NK = 1024, 32, 256, 128, 100000, 11
NCORES = 8
BLOC = B // NCORES  # 128
NGRP = BLOC // 4    # 32 groups of 4 batch items
NSC = 4             # super-chunks per pass (8 groups each)
GPS = NGRP // NSC   # 8 groups per super-chunk
SCCOLS = GPS * 128  # 1024 unique (b,q) cols per super-chunk
XCOLS = 2 * SCCOLS  # 2048 incl. both doc halves

DE_TILES = BLOC * 2           # 256 de gather tiles per pass
QE_TILES = NGRP               # 32 qe gather tiles per pass
TILES_PER_PASS = DE_TILES + QE_TILES  # 288
IDS_COLS = 2 * TILES_PER_PASS

_MUS = [-0.9, -0.7, -0.5, -0.3, -0.1, 0.1, 0.3, 0.5, 0.7, 0.9]  # sigma=0.1 bins


def _build_nc():
    import concourse.bass as bass
    import concourse.mybir as mybir
    import concourse.tile as tile
    from concourse import bacc
    from concourse.masks import make_identity

    f32 = mybir.dt.float32
    EXP = mybir.ActivationFunctionType.Exp
    SQUARE = mybir.ActivationFunctionType.Square
    LOG = mybir.ActivationFunctionType.Ln
    ADD = mybir.AluOpType.add
    AXX = mybir.AxisListType.X

    nc = bacc.Bacc(None, target_bir_lowering=False)
    with tile.TileContext(nc) as tc, ExitStack() as ctx:
        dram = ctx.enter_context(tc.tile_pool(name="dram", bufs=1, space="DRAM"))
        emb = dram.tile([VOCAB, EMBED], f32, kind="ExternalInput")
        ids = dram.tile([128, IDS_COLS], mybir.dt.int32, kind="ExternalInput")
        wvec = dram.tile([NK, 1], f32, kind="ExternalInput")
        out = dram.tile([1, BLOC], f32, kind="ExternalOutput")

        const = ctx.enter_context(tc.tile_pool(name="const", bufs=1))
        gde = ctx.enter_context(tc.tile_pool(name="gde", bufs=6))
        gqe = ctx.enter_context(tc.tile_pool(name="gqe", bufs=3))
        tps = ctx.enter_context(tc.tile_pool(name="tps", bufs=2, space="PSUM"))
        det = ctx.enter_context(tc.tile_pool(name="det", bufs=6))
        qet = ctx.enter_context(tc.tile_pool(name="qet", bufs=3))
        sps = ctx.enter_context(tc.tile_pool(name="sps", bufs=2, space="PSUM"))
        xp = ctx.enter_context(tc.tile_pool(name="xp", bufs=2))
        up = ctx.enter_context(tc.tile_pool(name="up", bufs=2))
        vp = ctx.enter_context(tc.tile_pool(name="vp", bufs=3))
        pp = ctx.enter_context(tc.tile_pool(name="pp", bufs=3))
        pooled = ctx.enter_context(tc.tile_pool(name="pooled", bufs=1, space="PSUM"))
        lgt = ctx.enter_context(tc.tile_pool(name="lgt", bufs=2, space="PSUM"))
        lp = ctx.enter_context(tc.tile_pool(name="lp", bufs=2))
        fp = ctx.enter_context(tc.tile_pool(name="fp", bufs=1))

        ids_sb = const.tile([128, IDS_COLS], mybir.dt.int32)
        nc.sync.dma_start(ids_sb[:], ids[:])
        w_sb = const.tile([NK, 1], f32)
        nc.sync.dma_start(w_sb[:], wvec[:])
        ident = const.tile([128, 128], f32)
        make_identity(nc, ident[:])
        # per-k ones-selector matrices: sel_k[:, j] = 1.0 iff j == k
        sels = []
        for k in range(NK):
            sel = const.tile([128, NK], f32, tag=f"sel{k}")
            nc.vector.memset(sel[:], 0.0)
            nc.vector.memset(sel[:, k : k + 1], 1.0)
            sels.append(sel)
        # bias constants as [128,1] APs (float biases need pre-registered
        # const APs; only 0.0/1.0 exist)
        bias_tiles = {}
        for val in sorted({-50.0 * mu * mu for mu in _MUS} | {-1000.0}):
            bt = const.tile([128, 1], f32, tag=f"bias{val}")
            nc.vector.memset(bt[:], val)
            bias_tiles[val] = bt

        f_sb = fp.tile([1, 2 * BLOC], f32)

        for p in range(2):
            idbase = p * TILES_PER_PASS
            for sc in range(NSC):
                X = xp.tile([128, XCOLS], f32, tag="X")
                # ---- gather + transpose + sim matmuls for 8 groups ----
                for gl in range(GPS):
                    g = sc * GPS + gl
                    qe = gqe.tile([128, 128], f32, tag="qe")
                    qcol = idbase + DE_TILES + g
                    nc.gpsimd.indirect_dma_start(
                        out=qe[:],
                        out_offset=None,
                        in_=emb[:],
                        in_offset=bass.IndirectOffsetOnAxis(
                            ap=ids_sb[:, qcol : qcol + 1], axis=0
                        ),
                    )
                    qeT_ps = tps.tile([128, 128], f32, tag="tps")
                    nc.tensor.transpose(qeT_ps[:], qe[:], ident[:])
                    qeT = qet.tile([128, 128], f32, tag="qeT")
                    nc.vector.tensor_copy(qeT[:], qeT_ps[:])

                    for h in range(2):
                        sim_ps = sps.tile([128, 128], f32, tag="sim")
                        for bs in range(4):
                            b = 4 * g + bs
                            dcol = idbase + 2 * b + h
                            de = gde.tile([128, 128], f32, tag="de")
                            nc.gpsimd.indirect_dma_start(
                                out=de[:],
                                out_offset=None,
                                in_=emb[:],
                                in_offset=bass.IndirectOffsetOnAxis(
                                    ap=ids_sb[:, dcol : dcol + 1], axis=0
                                ),
                            )
                            deT_ps = tps.tile([128, 128], f32, tag="tps")
                            nc.tensor.transpose(deT_ps[:], de[:], ident[:])
                            deT = det.tile([128, 128], f32, tag="deT")
                            nc.vector.tensor_copy(deT[:], deT_ps[:])
                            nc.tensor.matmul(
                                sim_ps[:, 32 * bs : 32 * bs + 32],
                                lhsT=deT[:],
                                rhs=qeT[:, 32 * bs : 32 * bs + 32],
                                start=True,
                                stop=True,
                            )
                        nc.scalar.copy(
                            X[:, h * SCCOLS + gl * 128 : h * SCCOLS + gl * 128 + 128],
                            sim_ps[:],
                        )

                # ---- histogram over this super-chunk ----
                T1 = up.tile([128, XCOLS], f32, tag="T1")
                nc.vector.tensor_mul(T1[:], X[:], X[:])
                U = up.tile([128, XCOLS], f32, tag="U")
                nc.scalar.activation(U[:], T1[:], EXP, scale=-50.0)

                pooled_ps = pooled.tile([NK, 1024], f32, tag="pool")

                for k in range(NK):
                    P = pp.tile([128, XCOLS], f32, tag="P")
                    if k < 10:
                        mu = _MUS[k]
                        V = vp.tile([128, XCOLS], f32, tag="V")
                        nc.scalar.activation(
                            V[:], X[:], EXP, scale=100.0 * mu,
                            bias=bias_tiles[-50.0 * mu * mu][:],
                        )
                        nc.vector.tensor_mul(P[:], U[:], V[:])
                    else:
                        V = vp.tile([128, XCOLS], f32, tag="V")
                        nc.scalar.activation(
                            V[:], X[:], SQUARE, scale=1000.0,
                            bias=bias_tiles[-1000.0][:],
                        )
                        nc.scalar.activation(P[:], V[:], EXP, scale=-0.5)
                    for blk in range(2):
                        for h in range(2):
                            nc.tensor.matmul(
                                pooled_ps[:, blk * 512 : blk * 512 + 512],
                                lhsT=sels[k][:],
                                rhs=P[
                                    :,
                                    h * SCCOLS + blk * 512 : h * SCCOLS + blk * 512 + 512,
                                ],
                                start=(k == 0 and h == 0),
                                stop=(k == NK - 1 and h == 1),
                            )

                # ---- log1p, mlp dot, query-sum ----
                L = lp.tile([NK, 1024], f32, tag="L")
                nc.scalar.activation(L[:, 0:512], pooled_ps[:, 0:512], LOG, bias=1.0)
                nc.scalar.activation(L[:, 512:1024], pooled_ps[:, 512:1024], LOG, bias=1.0)
                for blk in range(2):
                    logit_ps = lgt.tile([1, 512], f32, tag="logit")
                    nc.tensor.matmul(
                        logit_ps[:],
                        lhsT=w_sb[:],
                        rhs=L[:, blk * 512 : blk * 512 + 512],
                        start=True,
                        stop=True,
                    )
                    base = p * BLOC + sc * 32 + blk * 16
                    nc.vector.tensor_reduce(
                        f_sb[:, base : base + 16],
                        logit_ps[:].rearrange("o (b q) -> o b q", q=QLEN),
                        axis=AXX,
                        op=ADD,
                    )

        # ---- sigmoid(f1 - f2) ----
        diff = fp.tile([1, BLOC], f32)
        nc.vector.tensor_sub(diff[:], f_sb[:, 0:BLOC], f_sb[:, BLOC : 2 * BLOC])
        en = fp.tile([1, BLOC], f32)
        nc.scalar.activation(en[:], diff[:], EXP, scale=-1.0)
        enp1 = fp.tile([1, BLOC], f32)
        nc.vector.tensor_scalar_add(enp1[:], en[:], 1.0)
        sig = fp.tile([1, BLOC], f32)
        nc.vector.reciprocal(sig[:], enp1[:])
        nc.sync.dma_start(out[:], sig[:])

    nc.finalize()
    return nc, emb.name, ids.name, wvec.name, out.name


_CACHE = {}


def _get_nc():
    if "nc" not in _CACHE:
        _CACHE["nc"] = _build_nc()
    return _CACHE["nc"]


def _build_ids(query, doc):
    """query [128, 32] int, doc [128, 256] int -> ids [128, 288] int32.

    de tile (b, h): rows p = doc[b, 128h + p], at col 2b + h.
    qe tile g: rows p = query[4g + p // 32, p % 32], at col 512 + g.
    """
    ids = np.empty((128, TILES_PER_PASS), dtype=np.int32)
    ids[:, :DE_TILES] = (
        doc.reshape(BLOC, 2, 128).transpose(2, 0, 1).reshape(128, DE_TILES)
    )
    ids[:, DE_TILES:] = (
        query.reshape(NGRP, 4, QLEN).transpose(1, 2, 0).reshape(128, QE_TILES)
    )
    return ids


def kernel(emb, mlp_w, mlp_b, query1, doc1, query2, doc2):
    from concourse.bass_utils import run_bass_kernel_spmd

    emb = np.asarray(emb, dtype=np.float32)
    norms = np.sqrt((emb.astype(np.float64) ** 2).sum(axis=1, keepdims=True))
    emb_n = (emb.astype(np.float64) / norms).astype(np.float32)

    w = np.asarray(mlp_w, dtype=np.float32).reshape(NK, 1)
    q1 = np.asarray(query1).astype(np.int32)
    d1 = np.asarray(doc1).astype(np.int32)
    q2 = np.asarray(query2).astype(np.int32)
    d2 = np.asarray(doc2).astype(np.int32)

    nc, ename, iname, wname, oname = _get_nc()

    in_maps = []
    for c in range(NCORES):
        sl = slice(c * BLOC, (c + 1) * BLOC)
        idsv = np.concatenate(
            [_build_ids(q1[sl], d1[sl]), _build_ids(q2[sl], d2[sl])], axis=1
        )
        in_maps.append({ename: emb_n, iname: idsv, wname: w})

    trace = os.environ.get("KNRM_TRACE") == "1"
    res = run_bass_kernel_spmd(
        nc, in_maps, core_ids=list(range(NCORES)), trace=trace,
        trace_cores=[0] if trace else None,
    )
    global LAST_RESULT
    LAST_RESULT = res
    out = np.concatenate([res.results[c][oname].reshape(BLOC) for c in range(NCORES)])
    # mlp_b cancels in logits_1 - logits_2; output float32 [B, 1]
    return out.reshape(B, 1).astype(np.float32)
